# revision 16
# baseline (speedup 1.0000x reference)
"""Causal self-attention (B=2, T=2048, C=1024, H=16) on 8 TRN2 NeuronCores.

Sharding: core c -> batch b = c//4, head group hg = c%4 (4 heads/core).
Each core computes QKV for its 4 heads (column-parallel), causal attention,
and a row-parallel partial output projection [T, C]. The host sums the 4
partials per batch and adds the analytically-folded biases.

Device layouts (chosen so no on-chip transposes are ever needed):
  xt   [C=1024, T=2048] bf16   x[b] transposed (host-prepped)
  Q^T  [128, pair, T]   bf16   head pair packed on partitions (0-63 / 64-127)
  K^T  same
  vaug [128, tj, 4*66]  bf16   per head: col0 = ones, cols1-64 = V[tj block]
  S^T  [k=128, q<=512]  psum   row-packed K=64 matmuls, 2 heads concurrent
  P^T = exp(S^T)        bf16   (no max subtraction; scores are ~N(0,1))
  O^T  [65, 512] psum:  row0 = softmax denominator l, rows 1-64 = (P@V)^T
  yt   [128(h,d), T]    bf16   normalized attention output, feeds proj lhsT
"""

import sys

if "/opt/trn_rl_repo" not in sys.path:
    sys.path.insert(0, "/opt/trn_rl_repo")

import numpy as np
import ml_dtypes
from contextlib import ExitStack

import concourse.bass as bass
import concourse.mybir as mybir
import concourse.tile as tile
from concourse import bacc, bass_utils
from concourse.bass import ds, ts

BF = mybir.dt.bfloat16
F32 = mybir.dt.float32

B, T, C = 2, 2048, 1024
H, DK = 16, 64
P = 128
KC = C // P          # 8 contraction chunks over C
NTG = T // 512       # 4 t-groups of 512
NTJ = T // 128       # 16 t-chunks of 128
HPC = 4              # heads per core
VS = 66              # vaug per-head stride (col0 ones, 1-64 V, 65 pad)

NEG = -30000.0

# module-level knobs for test harness
TRACE = False
TRACE_KWARGS = {}
LAST_RESULTS = None


def _emit(ctx, tc, aps):
    nc = tc.nc
    xt, wq, wk, wv, bq, bk, wp, mask, out = (
        aps["xt"], aps["wq"], aps["wk"], aps["wv"], aps["bq"], aps["bk"],
        aps["wp"], aps["mask"], aps["out"],
    )

    consts = ctx.enter_context(tc.tile_pool(name="consts", bufs=1))
    bigs = ctx.enter_context(tc.tile_pool(name="bigs", bufs=1))
    temps = ctx.enter_context(tc.tile_pool(name="temps", bufs=3))
    ppool = ctx.enter_context(tc.tile_pool(name="ppool", bufs=3))
    psum = ctx.enter_context(tc.tile_pool(name="psum", bufs=1, space="PSUM"))
    dpool = ctx.enter_context(tc.tile_pool(name="dpool", bufs=2, space="DRAM"))

    # ---- load inputs to SBUF ----
    xts = bigs.tile([P, KC, T], BF)
    xtr = xt.rearrange("(k p) t -> p k t", p=P)
    for tg in range(NTG):
        nc.sync.dma_start(out=xts[:, :, ts(tg, 512)], in_=xtr[:, :, ts(tg, 512)])

    wqs = consts.tile([P, KC, 2 * P], BF)
    nc.sync.dma_start(out=wqs, in_=wq.rearrange("(k p) n -> p k n", p=P))
    wks = consts.tile([P, KC, 2 * P], BF)
    nc.sync.dma_start(out=wks, in_=wk.rearrange("(k p) n -> p k n", p=P))
    wvs = consts.tile([P, KC, 2 * P], BF)
    nc.sync.dma_start(out=wvs, in_=wv.rearrange("(k p) n -> p k n", p=P))
    wps = consts.tile([P, 2, C], BF)
    nc.sync.dma_start(out=wps, in_=wp.rearrange("(k p) n -> p k n", p=P))
    bqs = consts.tile([P, 2], F32)
    nc.sync.dma_start(out=bqs, in_=bq.rearrange("(m p) -> p m", p=P))
    bks = consts.tile([P, 2], F32)
    nc.sync.dma_start(out=bks, in_=bk.rearrange("(m p) -> p m", p=P))
    maskt = consts.tile([P, P], F32)
    nc.sync.dma_start(out=maskt, in_=mask)

    # ---- Q^T / K^T: [128(d pair-packed), pair, T] ----
    qt = bigs.tile([P, 2, T], BF)
    kt = bigs.tile([P, 2, T], BF)
    for wsrc, bsrc, dst in ((wqs, bqs, qt), (wks, bks, kt)):
        for m in range(2):
            for tg in range(NTG):
                pqk = psum.tile([P, 512], F32, tag="mm", bufs=2, name="pqk")
                for k in range(KC):
                    nc.tensor.matmul(
                        pqk,
                        lhsT=wsrc[:, k, ts(m, P)],
                        rhs=xts[:, k, ts(tg, 512)],
                        start=(k == 0),
                        stop=(k == KC - 1),
                    )
                nc.vector.tensor_add(
                    out=dst[:, m, ts(tg, 512)],
                    in0=pqk,
                    in1=bsrc[:, m : m + 1].to_broadcast([P, 512]),
                )

    # ---- V -> vaug [128, tj, 4*66] (col0 = ones) ----
    vaug = bigs.tile([P, NTJ, HPC * VS], BF)
    vaug4 = vaug.rearrange("p t (h c) -> p t h c", c=VS)
    nc.vector.memset(vaug4[:, :, :, DK : DK + 1], 1.0)
    for tj in range(NTJ):
        pv = psum.tile([P, 512], F32, tag="mm", bufs=2, name="pv")
        for k in range(KC):
            nc.tensor.matmul(
                pv[:, : 2 * P],
                lhsT=xts[:, k, ts(tj, P)],
                rhs=wvs[:, k, :],
                start=(k == 0),
                stop=(k == KC - 1),
            )
        nc.vector.tensor_copy(
            out=vaug4[:, tj, :, 0:DK],
            in_=pv[:, : 2 * P].rearrange("p (h d) -> p h d", d=DK),
        )

    # ---- attention ----
    yts = [bigs.tile([P, T], BF, name=f"yt{m}") for m in range(2)]
    for g in range(NTG):
        for m in range(2):
            po = [
                psum.tile([DK + 1, 512], F32, tag=f"o{h}", bufs=1, name=f"po{h}")
                for h in range(2)
            ]
            njc = 4 * g + 4
            for j in range(njc):
                jrel = j - 4 * g
                band = jrel >= 0
                ncols = 512 - 128 * jrel if band else 512
                qoff = g * 512 + (128 * jrel if band else 0)
                pss = []
                for h in range(2):
                    ps = psum.tile([P, 512], F32, tag=f"s{h}", bufs=2, name=f"ps{h}")
                    nc.tensor.matmul(
                        ps[:, :ncols],
                        lhsT=kt[h * DK : (h + 1) * DK, m, ts(j, P)],
                        rhs=qt[h * DK : (h + 1) * DK, m, ds(qoff, ncols)],
                        start=True,
                        stop=True,
                        tile_position=(h * DK, 0),
                    )
                    pss.append(ps)
                if band:
                    for h in range(2):
                        nc.vector.tensor_add(
                            out=pss[h][:, :P], in0=pss[h][:, :P], in1=maskt
                        )
                for h in range(2):
                    pt = ppool.tile([P, 512], BF, tag=f"p{h}", name=f"pt{h}")
                    nc.scalar.activation(
                        pt[:, :ncols],
                        pss[h][:, :ncols],
                        mybir.ActivationFunctionType.Exp,
                    )
                    co = 128 * jrel if band else 0
                    nc.tensor.matmul(
                        po[h][:, co : co + ncols],
                        lhsT=vaug4[:, j, 2 * m + h, : DK + 1],
                        rhs=pt[:, :ncols],
                        start=(j == 0),
                        stop=(j == njc - 1),
                        skip_group_check=True,
                    )
            # finalize: normalize rows 0-63 by row 64 (the exp-sum), place into yt
            for h in range(2):
                rr = temps.tile([P, 512], F32, tag="rr", name="rr")
                nc.vector.reciprocal(out=rr[DK : DK + 1, :], in_=po[h][DK : DK + 1, :])
                dscr = dpool.tile([512], F32, tag="dscr", name="dscr")
                nc.sync.dma_start(out=dscr, in_=rr[DK : DK + 1, :])
                rb = temps.tile([P, 512], F32, tag="rb", name="rb")
                nc.gpsimd.dma_start(
                    out=rb[:DK, :],
                    in_=bass.AP(
                        tensor=dscr.tensor,
                        offset=dscr.offset,
                        ap=[[0, DK]] + list(dscr.ap),
                    ),
                )
                stg = temps.tile([P, 512], BF, tag="stg", name="stg")
                nc.vector.tensor_mul(
                    out=stg[:DK, :],
                    in0=po[h][:DK, :],
                    in1=rb[:DK, :],
                )
                nc.sync.dma_start(
                    out=yts[m][h * DK : (h + 1) * DK, ts(g, 512)],
                    in_=stg[:DK, :],
                )

    # ---- output projection: partial [T, C] ----
    for tj in range(NTJ):
        pps = [
            psum.tile([P, 512], F32, tag="mm", bufs=2, name=f"pp{n}") for n in range(2)
        ]
        for kc in range(2):
            for n in range(2):
                nc.tensor.matmul(
                    pps[n],
                    lhsT=yts[kc][:, ts(tj, P)],
                    rhs=wps[:, kc, ts(n, 512)],
                    start=(kc == 0),
                    stop=(kc == 1),
                )
        for n in range(2):
            ostg = temps.tile([P, 512], F32, tag="ostg", name="ostg")
            nc.vector.tensor_copy(out=ostg, in_=pps[n])
            nc.sync.dma_start(out=out[ts(tj, P), ts(n, 512)], in_=ostg)


_NC_CACHE = None


def build():
    global _NC_CACHE
    if _NC_CACHE is not None:
        return _NC_CACHE
    nc = bacc.Bacc("TRN2", target_bir_lowering=False, debug=False, num_devices=8)
    aps = {
        "xt": nc.dram_tensor("xt", [C, T], BF, kind="ExternalInput").ap(),
        "wq": nc.dram_tensor("wq", [C, 2 * P], BF, kind="ExternalInput").ap(),
        "wk": nc.dram_tensor("wk", [C, 2 * P], BF, kind="ExternalInput").ap(),
        "wv": nc.dram_tensor("wv", [C, 2 * P], BF, kind="ExternalInput").ap(),
        "bq": nc.dram_tensor("bq", [2 * P], F32, kind="ExternalInput").ap(),
        "bk": nc.dram_tensor("bk", [2 * P], F32, kind="ExternalInput").ap(),
        "wp": nc.dram_tensor("wp", [2 * P, C], BF, kind="ExternalInput").ap(),
        "mask": nc.dram_tensor("mask", [P, P], F32, kind="ExternalInput").ap(),
        "out": nc.dram_tensor("out", [T, C], F32, kind="ExternalOutput").ap(),
    }
    with tile.TileContext(nc) as tc:
        with ExitStack() as ctx:
            _emit(ctx, tc, aps)
    nc.compile()
    _NC_CACHE = nc
    return nc


def make_in_maps(x, Wqkv, bqkv, Wproj):
    """Host-side sharding/layout prep. Returns per-core input dicts."""
    bf = ml_dtypes.bfloat16
    scale = np.float32(1.0 / np.sqrt(DK))
    maskv = np.where(
        np.arange(P)[None, :] >= np.arange(P)[:, None], 0.0, NEG
    ).astype(np.float32)
    xts = [np.ascontiguousarray(x[b].T).astype(bf) for b in range(B)]
    in_maps = []
    for c in range(8):
        b, hg = divmod(c, 4)
        lo = hg * HPC * DK
        sl = slice(lo, lo + HPC * DK)
        in_maps.append(
            {
                "xt": xts[b],
                "wq": np.ascontiguousarray(Wqkv[:, 0 * C :][:, sl] * scale).astype(bf),
                "wk": np.ascontiguousarray(Wqkv[:, 1 * C :][:, sl]).astype(bf),
                "wv": np.ascontiguousarray(Wqkv[:, 2 * C :][:, sl]).astype(bf),
                "bq": np.ascontiguousarray(bqkv[0 * C :][sl] * scale).astype(np.float32),
                "bk": np.ascontiguousarray(bqkv[1 * C :][sl]).astype(np.float32),
                "wp": np.ascontiguousarray(Wproj[sl, :]).astype(bf),
                "mask": maskv,
            }
        )
    return in_maps


def gather(outs, bqkv, Wproj, bproj):
    """Sum per-core partials per batch; fold V-bias + proj-bias analytically."""
    bv = bqkv[2 * C :].astype(np.float32)
    bp_eff = (bproj.astype(np.float32) + bv @ Wproj.astype(np.float32)).astype(
        np.float32
    )
    y = np.empty((B, T, C), np.float32)
    for b in range(B):
        acc = outs[b * 4 + 0].astype(np.float32).copy()
        for hg in range(1, 4):
            acc += outs[b * 4 + hg]
        y[b] = acc + bp_eff[None, :]
    return y


def kernel(x, Wqkv, bqkv, Wproj, bproj):
    global LAST_RESULTS
    x = np.asarray(x, dtype=np.float32)
    Wqkv = np.asarray(Wqkv, dtype=np.float32)
    bqkv = np.asarray(bqkv, dtype=np.float32)
    Wproj = np.asarray(Wproj, dtype=np.float32)
    bproj = np.asarray(bproj, dtype=np.float32)

    nc = build()
    in_maps = make_in_maps(x, Wqkv, bqkv, Wproj)
    try:
        res = bass_utils.run_bass_kernel_spmd(
            nc,
            in_maps,
            core_ids=list(range(8)),
            trace=TRACE,
            **TRACE_KWARGS,
        )
    except Exception:
        if not TRACE:
            raise
        import traceback

        traceback.print_exc()
        print("traced run failed; retrying without trace", file=sys.stderr)
        res = bass_utils.run_bass_kernel_spmd(nc, in_maps, core_ids=list(range(8)))
    LAST_RESULTS = res
    outs = [res.results[c]["out"] for c in range(8)]
    return gather(outs, bqkv, Wproj, bproj)


# revision 19
# speedup vs baseline: 1.3196x; 1.3196x over previous
"""Causal self-attention (B=2, T=2048, C=1024, H=16) on 8 TRN2 NeuronCores.

Sharding: core c -> batch b = c//4, head group hg = c%4 (4 heads/core).
Each core computes QKV for its 4 heads (column-parallel), causal attention,
and a row-parallel partial output projection [T, C]. The host sums the 4
partials per batch and adds the analytically-folded biases.

Device layouts (chosen so no on-chip transposes are ever needed):
  xt   [C=1024, T=2048] bf16   x[b] transposed (host-prepped)
  Q^T  [128, pair, T]   bf16   head pair packed on partitions (0-63 / 64-127)
  K^T  same
  vaug [128, tj, 4*66]  bf16   per head: col0 = ones, cols1-64 = V[tj block]
  S^T  [k=128, q<=512]  psum   row-packed K=64 matmuls, 2 heads concurrent
  P^T = exp(S^T)        bf16   (no max subtraction; scores are ~N(0,1))
  O^T  [65, 512] psum:  row0 = softmax denominator l, rows 1-64 = (P@V)^T
  yt   [128(h,d), T]    bf16   normalized attention output, feeds proj lhsT
"""

import sys

if "/opt/trn_rl_repo" not in sys.path:
    sys.path.insert(0, "/opt/trn_rl_repo")

import numpy as np
import ml_dtypes
from contextlib import ExitStack

import concourse.bass as bass
import concourse.mybir as mybir
import concourse.tile as tile
from concourse import bacc, bass_utils
from concourse.bass import ds, ts

BF = mybir.dt.bfloat16
F32 = mybir.dt.float32

B, T, C = 2, 2048, 1024
H, DK = 16, 64
P = 128
KC = C // P          # 8 contraction chunks over C
NTG = T // 512       # 4 t-groups of 512
NTJ = T // 128       # 16 t-chunks of 128
HPC = 4              # heads per core
VS = 66              # vaug per-head stride (col0 ones, 1-64 V, 65 pad)

NEG = -30000.0

# module-level knobs for test harness
TRACE = False
TRACE_KWARGS = {}
LAST_RESULTS = None


def _emit(ctx, tc, aps):
    nc = tc.nc
    xt, wq, wk, wv, bq, bk, wp, mask, out = (
        aps["xt"], aps["wq"], aps["wk"], aps["wv"], aps["bq"], aps["bk"],
        aps["wp"], aps["mask"], aps["out"],
    )

    consts = ctx.enter_context(tc.tile_pool(name="consts", bufs=1))
    bigs = ctx.enter_context(tc.tile_pool(name="bigs", bufs=1))
    temps = ctx.enter_context(tc.tile_pool(name="temps", bufs=3))
    ppool = ctx.enter_context(tc.tile_pool(name="ppool", bufs=3))
    psum = ctx.enter_context(tc.tile_pool(name="psum", bufs=1, space="PSUM"))
    dpool = ctx.enter_context(tc.tile_pool(name="dpool", bufs=2, space="DRAM"))

    # ---- load inputs to SBUF ----
    xts = bigs.tile([P, KC, T], BF)
    xtr = xt.rearrange("(k p) t -> p k t", p=P)
    for tg in range(NTG):
        nc.sync.dma_start(out=xts[:, :, ts(tg, 512)], in_=xtr[:, :, ts(tg, 512)])

    wqs = consts.tile([P, KC, 2 * P], BF)
    nc.sync.dma_start(out=wqs, in_=wq.rearrange("(k p) n -> p k n", p=P))
    wks = consts.tile([P, KC, 2 * P], BF)
    nc.sync.dma_start(out=wks, in_=wk.rearrange("(k p) n -> p k n", p=P))
    wvs = consts.tile([P, KC, 2 * P], BF)
    nc.sync.dma_start(out=wvs, in_=wv.rearrange("(k p) n -> p k n", p=P))
    wps = consts.tile([P, 2, C], BF)
    nc.sync.dma_start(out=wps, in_=wp.rearrange("(k p) n -> p k n", p=P))
    bqs = consts.tile([P, 2], F32)
    nc.sync.dma_start(out=bqs, in_=bq.rearrange("(m p) -> p m", p=P))
    bks = consts.tile([P, 2], F32)
    nc.sync.dma_start(out=bks, in_=bk.rearrange("(m p) -> p m", p=P))
    maskt = consts.tile([P, P], F32)
    nc.sync.dma_start(out=maskt, in_=mask)

    # ---- Q^T / K^T: [128(d pair-packed), pair, T] ----
    qt = bigs.tile([P, 2, T], BF)
    kt = bigs.tile([P, 2, T], BF)

    def emit_qk(m):
        for wsrc, bsrc, dst in ((wqs, bqs, qt), (wks, bks, kt)):
            for tg in range(NTG):
                pqk = psum.tile([P, 512], F32, tag="mm", bufs=2, name="pqk")
                for k in range(KC):
                    nc.tensor.matmul(
                        pqk,
                        lhsT=wsrc[:, k, ts(m, P)],
                        rhs=xts[:, k, ts(tg, 512)],
                        start=(k == 0),
                        stop=(k == KC - 1),
                    )
                nc.vector.tensor_add(
                    out=dst[:, m, ts(tg, 512)],
                    in0=pqk,
                    in1=bsrc[:, m : m + 1].to_broadcast([P, 512]),
                )

    # ---- V -> vaug [128, tj, 4*66] (col DK = ones) ----
    vaug = bigs.tile([P, NTJ, HPC * VS], BF)
    vaug4 = vaug.rearrange("p t (h c) -> p t h c", c=VS)

    def emit_v():
        nc.vector.memset(vaug4[:, :, :, DK : DK + 1], 1.0)
        for tj in range(NTJ):
            pv = psum.tile([P, 512], F32, tag="mm", bufs=2, name="pv")
            for k in range(KC):
                nc.tensor.matmul(
                    pv[:, : 2 * P],
                    lhsT=xts[:, k, ts(tj, P)],
                    rhs=wvs[:, k, :],
                    start=(k == 0),
                    stop=(k == KC - 1),
                )
            nc.vector.tensor_copy(
                out=vaug4[:, tj, :, 0:DK],
                in_=pv[:, : 2 * P].rearrange("p (h d) -> p h d", d=DK),
            )

    # ---- attention ----
    yts = [bigs.tile([P, T], BF, name=f"yt{m}") for m in range(2)]

    def emit_attn(m, g):
        po = [
            psum.tile([DK + 1, 512], F32, tag=f"o{h}", bufs=1, name=f"po{h}")
            for h in range(2)
        ]
        njc = 4 * g + 4
        for j in range(njc):
            jrel = j - 4 * g
            band = jrel >= 0
            ncols = 512 - 128 * jrel if band else 512
            qoff = g * 512 + (128 * jrel if band else 0)
            pss = []
            for h in range(2):
                ps = psum.tile([P, 512], F32, tag=f"s{h}", bufs=2, name=f"ps{h}")
                nc.tensor.matmul(
                    ps[:, :ncols],
                    lhsT=kt[h * DK : (h + 1) * DK, m, ts(j, P)],
                    rhs=qt[h * DK : (h + 1) * DK, m, ds(qoff, ncols)],
                    start=True,
                    stop=True,
                    tile_position=(h * DK, 0),
                )
                pss.append(ps)
            if band:
                for h in range(2):
                    nc.vector.tensor_add(
                        out=pss[h][:, :P], in0=pss[h][:, :P], in1=maskt
                    )
            for h in range(2):
                pt = ppool.tile([P, 512], BF, tag=f"p{h}", name=f"pt{h}")
                nc.scalar.activation(
                    pt[:, :ncols],
                    pss[h][:, :ncols],
                    mybir.ActivationFunctionType.Exp,
                )
                co = 128 * jrel if band else 0
                nc.tensor.matmul(
                    po[h][:, co : co + ncols],
                    lhsT=vaug4[:, j, 2 * m + h, : DK + 1],
                    rhs=pt[:, :ncols],
                    start=(j == 0),
                    stop=(j == njc - 1),
                    skip_group_check=True,
                )
        # finalize: copy O^T off PSUM fast, then normalize rows 0-63 by the
        # broadcast exp-sum (row 64) and place into yt
        for h in range(2):
            oc = temps.tile([P, 512], F32, tag="oc", name="oc")
            nc.vector.tensor_copy(out=oc[: DK + 1, :], in_=po[h])
            dscr = dpool.tile([512], F32, tag="dscr", name="dscr")
            nc.sync.dma_start(out=dscr, in_=oc[DK : DK + 1, :])
            rbl = temps.tile([P, 512], F32, tag="rbl", name="rbl")
            nc.gpsimd.dma_start(
                out=rbl[:DK, :],
                in_=bass.AP(
                    tensor=dscr.tensor,
                    offset=dscr.offset,
                    ap=[[0, DK]] + list(dscr.ap),
                ),
            )
            rb = temps.tile([P, 512], F32, tag="rb", name="rb")
            nc.vector.reciprocal_approx_fast(out=rb[:DK, :], in_=rbl[:DK, :])
            stg = temps.tile([P, 512], BF, tag="stg", name="stg")
            nc.vector.tensor_mul(
                out=stg[:DK, :],
                in0=oc[:DK, :],
                in1=rb[:DK, :],
            )
            nc.sync.dma_start(
                out=yts[m][h * DK : (h + 1) * DK, ts(g, 512)],
                in_=stg[:DK, :],
            )

    # ---- output projection: partial [T, C] for one t-group of 4 chunks ----
    def emit_proj(g):
        for tj in range(4 * g, 4 * g + 4):
            pps = [
                psum.tile([P, 512], F32, tag="mm", bufs=2, name=f"pp{n}")
                for n in range(2)
            ]
            for kc in range(2):
                for n in range(2):
                    nc.tensor.matmul(
                        pps[n],
                        lhsT=yts[kc][:, ts(tj, P)],
                        rhs=wps[:, kc, ts(n, 512)],
                        start=(kc == 0),
                        stop=(kc == 1),
                    )
            for n in range(2):
                ostg = temps.tile([P, 512], F32, tag="ostg", name="ostg")
                nc.vector.tensor_copy(out=ostg, in_=pps[n])
                nc.sync.dma_start(out=out[ts(tj, P), ts(n, 512)], in_=ostg)

    # ---- schedule: weave PE-dense QKV/proj work between ACT-gated attention ----
    emit_qk(0)
    emit_v()
    emit_attn(0, 0)
    emit_qk(1)
    emit_attn(1, 0)
    for g in range(1, NTG):
        emit_attn(0, g)
        emit_attn(1, g)
        emit_proj(g - 1)
    emit_proj(NTG - 1)


_NC_CACHE = None


def build():
    global _NC_CACHE
    if _NC_CACHE is not None:
        return _NC_CACHE
    nc = bacc.Bacc("TRN2", target_bir_lowering=False, debug=False, num_devices=8)
    aps = {
        "xt": nc.dram_tensor("xt", [C, T], BF, kind="ExternalInput").ap(),
        "wq": nc.dram_tensor("wq", [C, 2 * P], BF, kind="ExternalInput").ap(),
        "wk": nc.dram_tensor("wk", [C, 2 * P], BF, kind="ExternalInput").ap(),
        "wv": nc.dram_tensor("wv", [C, 2 * P], BF, kind="ExternalInput").ap(),
        "bq": nc.dram_tensor("bq", [2 * P], F32, kind="ExternalInput").ap(),
        "bk": nc.dram_tensor("bk", [2 * P], F32, kind="ExternalInput").ap(),
        "wp": nc.dram_tensor("wp", [2 * P, C], BF, kind="ExternalInput").ap(),
        "mask": nc.dram_tensor("mask", [P, P], F32, kind="ExternalInput").ap(),
        "out": nc.dram_tensor("out", [T, C], F32, kind="ExternalOutput").ap(),
    }
    with tile.TileContext(nc) as tc:
        with ExitStack() as ctx:
            _emit(ctx, tc, aps)
    nc.compile()
    _NC_CACHE = nc
    return nc


def make_in_maps(x, Wqkv, bqkv, Wproj):
    """Host-side sharding/layout prep. Returns per-core input dicts."""
    bf = ml_dtypes.bfloat16
    scale = np.float32(1.0 / np.sqrt(DK))
    maskv = np.where(
        np.arange(P)[None, :] >= np.arange(P)[:, None], 0.0, NEG
    ).astype(np.float32)
    xts = [np.ascontiguousarray(x[b].T).astype(bf) for b in range(B)]
    in_maps = []
    for c in range(8):
        b, hg = divmod(c, 4)
        lo = hg * HPC * DK
        sl = slice(lo, lo + HPC * DK)
        in_maps.append(
            {
                "xt": xts[b],
                "wq": np.ascontiguousarray(Wqkv[:, 0 * C :][:, sl] * scale).astype(bf),
                "wk": np.ascontiguousarray(Wqkv[:, 1 * C :][:, sl]).astype(bf),
                "wv": np.ascontiguousarray(Wqkv[:, 2 * C :][:, sl]).astype(bf),
                "bq": np.ascontiguousarray(bqkv[0 * C :][sl] * scale).astype(np.float32),
                "bk": np.ascontiguousarray(bqkv[1 * C :][sl]).astype(np.float32),
                "wp": np.ascontiguousarray(Wproj[sl, :]).astype(bf),
                "mask": maskv,
            }
        )
    return in_maps


def gather(outs, bqkv, Wproj, bproj):
    """Sum per-core partials per batch; fold V-bias + proj-bias analytically."""
    bv = bqkv[2 * C :].astype(np.float32)
    bp_eff = (bproj.astype(np.float32) + bv @ Wproj.astype(np.float32)).astype(
        np.float32
    )
    y = np.empty((B, T, C), np.float32)
    for b in range(B):
        acc = outs[b * 4 + 0].astype(np.float32).copy()
        for hg in range(1, 4):
            acc += outs[b * 4 + hg]
        y[b] = acc + bp_eff[None, :]
    return y


def kernel(x, Wqkv, bqkv, Wproj, bproj):
    global LAST_RESULTS
    x = np.asarray(x, dtype=np.float32)
    Wqkv = np.asarray(Wqkv, dtype=np.float32)
    bqkv = np.asarray(bqkv, dtype=np.float32)
    Wproj = np.asarray(Wproj, dtype=np.float32)
    bproj = np.asarray(bproj, dtype=np.float32)

    nc = build()
    in_maps = make_in_maps(x, Wqkv, bqkv, Wproj)
    try:
        res = bass_utils.run_bass_kernel_spmd(
            nc,
            in_maps,
            core_ids=list(range(8)),
            trace=TRACE,
            **TRACE_KWARGS,
        )
    except Exception:
        if not TRACE:
            raise
        import traceback

        traceback.print_exc()
        print("traced run failed; retrying without trace", file=sys.stderr)
        res = bass_utils.run_bass_kernel_spmd(nc, in_maps, core_ids=list(range(8)))
    LAST_RESULTS = res
    outs = [res.results[c]["out"] for c in range(8)]
    return gather(outs, bqkv, Wproj, bproj)


# revision 23
# speedup vs baseline: 1.5573x; 1.1801x over previous
"""Causal self-attention (B=2, T=2048, C=1024, H=16) on 8 TRN2 NeuronCores.

Sharding: core c -> batch b = c//4, head group hg = c%4 (4 heads/core).
Each core computes QKV for its 4 heads (column-parallel), causal attention,
and a row-parallel partial output projection [T, C]. The host sums the 4
partials per batch and adds the analytically-folded biases.

Device layouts (chosen so no on-chip transposes are ever needed):
  xt   [C=1024, T=2048] bf16   x[b] transposed (host-prepped)
  Q^T  [128, pair, T]   bf16   head pair packed on partitions (0-63 / 64-127)
  K^T  same
  vaug [128, tj, 4*66]  bf16   per head: col0 = ones, cols1-64 = V[tj block]
  S^T  [k=128, q<=512]  psum   row-packed K=64 matmuls, 2 heads concurrent
  P^T = exp(S^T)        bf16   (no max subtraction; scores are ~N(0,1))
  O^T  [65, 512] psum:  row0 = softmax denominator l, rows 1-64 = (P@V)^T
  yt   [128(h,d), T]    bf16   normalized attention output, feeds proj lhsT
"""

import sys

if "/opt/trn_rl_repo" not in sys.path:
    sys.path.insert(0, "/opt/trn_rl_repo")

import numpy as np
import ml_dtypes
from contextlib import ExitStack

import concourse.bass as bass
import concourse.mybir as mybir
import concourse.tile as tile
from concourse import bacc, bass_utils
from concourse.bass import ds, ts

BF = mybir.dt.bfloat16
F32 = mybir.dt.float32

B, T, C = 2, 2048, 1024
H, DK = 16, 64
P = 128
KC = C // P          # 8 contraction chunks over C
NTG = T // 512       # 4 t-groups of 512
NTJ = T // 128       # 16 t-chunks of 128
HPC = 4              # heads per core
VS = 66              # vaug per-head stride (col0 ones, 1-64 V, 65 pad)

NEG = -30000.0

# module-level knobs for test harness
TRACE = False
TRACE_KWARGS = {}
LAST_RESULTS = None


def _emit(ctx, tc, aps):
    nc = tc.nc
    xt, wq, wk, wv, bq, bk, wp, mask, out = (
        aps["xt"], aps["wq"], aps["wk"], aps["wv"], aps["bq"], aps["bk"],
        aps["wp"], aps["mask"], aps["out"],
    )

    consts = ctx.enter_context(tc.tile_pool(name="consts", bufs=1))
    bigs = ctx.enter_context(tc.tile_pool(name="bigs", bufs=1))
    temps = ctx.enter_context(tc.tile_pool(name="temps", bufs=4))
    ppool = ctx.enter_context(tc.tile_pool(name="ppool", bufs=4))
    psum = ctx.enter_context(tc.tile_pool(name="psum", bufs=1, space="PSUM"))
    dpool = ctx.enter_context(tc.tile_pool(name="dpool", bufs=2, space="DRAM"))

    # ---- load inputs to SBUF ----
    xts = bigs.tile([P, KC, T], BF)
    xtr = xt.rearrange("(k p) t -> p k t", p=P)
    for tg in range(NTG):
        nc.sync.dma_start(out=xts[:, :, ts(tg, 512)], in_=xtr[:, :, ts(tg, 512)])

    wqs = consts.tile([P, KC, 2 * P], BF)
    nc.sync.dma_start(out=wqs, in_=wq.rearrange("(k p) n -> p k n", p=P))
    wks = consts.tile([P, KC, 2 * P], BF)
    nc.sync.dma_start(out=wks, in_=wk.rearrange("(k p) n -> p k n", p=P))
    wvs = consts.tile([P, KC, 2 * P], BF)
    nc.sync.dma_start(out=wvs, in_=wv.rearrange("(k p) n -> p k n", p=P))
    wps = consts.tile([P, 2, C], BF)
    nc.sync.dma_start(out=wps, in_=wp.rearrange("(k p) n -> p k n", p=P))
    bqs = consts.tile([P, 2], F32)
    nc.sync.dma_start(out=bqs, in_=bq.rearrange("(m p) -> p m", p=P))
    bks = consts.tile([P, 2], F32)
    nc.sync.dma_start(out=bks, in_=bk.rearrange("(m p) -> p m", p=P))
    maskt = consts.tile([P, P], F32)
    nc.sync.dma_start(out=maskt, in_=mask)

    # ---- Q^T / K^T: [128(d pair-packed), pair, T] ----
    qt = bigs.tile([P, 2, T], BF)
    kt = bigs.tile([P, 2, T], BF)

    def emit_qk(m, tg):
        for wsrc, bsrc, dst in ((wqs, bqs, qt), (wks, bks, kt)):
            pqk = psum.tile([P, 512], F32, tag="mm", bufs=2, name="pqk")
            for k in range(KC):
                nc.tensor.matmul(
                    pqk,
                    lhsT=wsrc[:, k, ts(m, P)],
                    rhs=xts[:, k, ts(tg, 512)],
                    start=(k == 0),
                    stop=(k == KC - 1),
                )
            nc.vector.tensor_add(
                out=dst[:, m, ts(tg, 512)],
                in0=pqk,
                in1=bsrc[:, m : m + 1].to_broadcast([P, 512]),
            )

    # ---- V -> vaug [128, tj, 4*66] (col DK = ones) ----
    vaug = bigs.tile([P, NTJ, HPC * VS], BF)
    vaug4 = vaug.rearrange("p t (h c) -> p t h c", c=VS)

    def emit_v(g):
        for tj in range(4 * g, 4 * g + 4):
            pv = psum.tile([P, 512], F32, tag="mm", bufs=2, name="pv")
            for k in range(KC):
                nc.tensor.matmul(
                    pv[:, : 2 * P],
                    lhsT=xts[:, k, ts(tj, P)],
                    rhs=wvs[:, k, :],
                    start=(k == 0),
                    stop=(k == KC - 1),
                )
            nc.vector.tensor_copy(
                out=vaug4[:, tj, :, 0:DK],
                in_=pv[:, : 2 * P].rearrange("p (h d) -> p h d", d=DK),
            )

    # ---- attention ----
    yts = [bigs.tile([P, T], BF, name=f"yt{m}") for m in range(2)]

    def emit_attn(m, g):
        po = [
            psum.tile([DK + 1, 512], F32, tag=f"o{h}", bufs=1, name=f"po{h}")
            for h in range(2)
        ]
        njc = 4 * g + 4
        for j in range(njc):
            jrel = j - 4 * g
            band = jrel >= 0
            ncols = 512 - 128 * jrel if band else 512
            qoff = g * 512 + (128 * jrel if band else 0)
            pss = []
            for h in range(2):
                ps = psum.tile([P, 512], F32, tag=f"s{h}", bufs=2, name=f"ps{h}")
                nc.tensor.matmul(
                    ps[:, :ncols],
                    lhsT=kt[h * DK : (h + 1) * DK, m, ts(j, P)],
                    rhs=qt[h * DK : (h + 1) * DK, m, ds(qoff, ncols)],
                    start=True,
                    stop=True,
                    tile_position=(h * DK, 0),
                )
                pss.append(ps)
            if band:
                for h in range(2):
                    nc.vector.tensor_add(
                        out=pss[h][:, :P], in0=pss[h][:, :P], in1=maskt
                    )
            for h in range(2):
                pt = ppool.tile([P, 512], BF, tag=f"p{h}", name=f"pt{h}")
                nc.scalar.activation(
                    pt[:, :ncols],
                    pss[h][:, :ncols],
                    mybir.ActivationFunctionType.Exp,
                )
                co = 128 * jrel if band else 0
                nc.tensor.matmul(
                    po[h][:, co : co + ncols],
                    lhsT=vaug4[:, j, 2 * m + h, : DK + 1],
                    rhs=pt[:, :ncols],
                    start=(j == 0),
                    stop=(j == njc - 1),
                    skip_group_check=True,
                )
        # finalize: copy O^T off PSUM fast, then normalize rows 0-63 by the
        # broadcast exp-sum (row 64) and place into yt
        for h in range(2):
            oc = temps.tile([P, 512], F32, tag="oc", name="oc")
            nc.vector.tensor_copy(out=oc[: DK + 1, :], in_=po[h])
            dscr = dpool.tile([512], F32, tag="dscr", name="dscr")
            nc.sync.dma_start(out=dscr, in_=oc[DK : DK + 1, :])
            rbl = temps.tile([P, 512], F32, tag="rbl", name="rbl")
            nc.gpsimd.dma_start(
                out=rbl[:DK, :],
                in_=bass.AP(
                    tensor=dscr.tensor,
                    offset=dscr.offset,
                    ap=[[0, DK]] + list(dscr.ap),
                ),
            )
            rb = temps.tile([P, 512], F32, tag="rb", name="rb")
            nc.vector.reciprocal_approx_fast(out=rb[:DK, :], in_=rbl[:DK, :])
            stg = temps.tile([P, 512], BF, tag="stg", name="stg")
            nc.vector.tensor_mul(
                out=stg[:DK, :],
                in0=oc[:DK, :],
                in1=rb[:DK, :],
            )
            nc.sync.dma_start(
                out=yts[m][h * DK : (h + 1) * DK, ts(g, 512)],
                in_=stg[:DK, :],
            )

    # ---- output projection: partial [T, C] for one t-group of 4 chunks ----
    def emit_proj(g):
        for tj in range(4 * g, 4 * g + 4):
            pps = [
                psum.tile([P, 512], F32, tag="mm", bufs=2, name=f"pp{n}")
                for n in range(2)
            ]
            for kc in range(2):
                for n in range(2):
                    nc.tensor.matmul(
                        pps[n],
                        lhsT=yts[kc][:, ts(tj, P)],
                        rhs=wps[:, kc, ts(n, 512)],
                        start=(kc == 0),
                        stop=(kc == 1),
                    )
            for n in range(2):
                ostg = temps.tile([P, 512], F32, tag="ostg", name="ostg")
                nc.vector.tensor_copy(out=ostg, in_=pps[n])
                nc.sync.dma_start(out=out[ts(tj, P), ts(n, 512)], in_=ostg)

    # ---- schedule: pipeline by q-group, weaving PE-dense QKV/proj work
    # between ACT-gated attention so both engine queues stay fed ----
    nc.vector.memset(vaug4[:, :, :, DK : DK + 1], 1.0)
    for g in range(NTG):
        emit_qk(0, g)
        emit_v(g)
        emit_attn(0, g)
        emit_qk(1, g)
        emit_attn(1, g)
        if g >= 1:
            emit_proj(g - 1)
    emit_proj(NTG - 1)


_NC_CACHE = None


def build():
    global _NC_CACHE
    if _NC_CACHE is not None:
        return _NC_CACHE
    nc = bacc.Bacc("TRN2", target_bir_lowering=False, debug=False, num_devices=8)
    aps = {
        "xt": nc.dram_tensor("xt", [C, T], BF, kind="ExternalInput").ap(),
        "wq": nc.dram_tensor("wq", [C, 2 * P], BF, kind="ExternalInput").ap(),
        "wk": nc.dram_tensor("wk", [C, 2 * P], BF, kind="ExternalInput").ap(),
        "wv": nc.dram_tensor("wv", [C, 2 * P], BF, kind="ExternalInput").ap(),
        "bq": nc.dram_tensor("bq", [2 * P], F32, kind="ExternalInput").ap(),
        "bk": nc.dram_tensor("bk", [2 * P], F32, kind="ExternalInput").ap(),
        "wp": nc.dram_tensor("wp", [2 * P, C], BF, kind="ExternalInput").ap(),
        "mask": nc.dram_tensor("mask", [P, P], F32, kind="ExternalInput").ap(),
        "out": nc.dram_tensor("out", [T, C], F32, kind="ExternalOutput").ap(),
    }
    with tile.TileContext(nc) as tc:
        with ExitStack() as ctx:
            _emit(ctx, tc, aps)
    nc.compile()
    _NC_CACHE = nc
    return nc


def make_in_maps(x, Wqkv, bqkv, Wproj):
    """Host-side sharding/layout prep. Returns per-core input dicts."""
    bf = ml_dtypes.bfloat16
    scale = np.float32(1.0 / np.sqrt(DK))
    maskv = np.where(
        np.arange(P)[None, :] >= np.arange(P)[:, None], 0.0, NEG
    ).astype(np.float32)
    xts = [np.ascontiguousarray(x[b].T).astype(bf) for b in range(B)]
    in_maps = []
    for c in range(8):
        b, hg = divmod(c, 4)
        lo = hg * HPC * DK
        sl = slice(lo, lo + HPC * DK)
        in_maps.append(
            {
                "xt": xts[b],
                "wq": np.ascontiguousarray(Wqkv[:, 0 * C :][:, sl] * scale).astype(bf),
                "wk": np.ascontiguousarray(Wqkv[:, 1 * C :][:, sl]).astype(bf),
                "wv": np.ascontiguousarray(Wqkv[:, 2 * C :][:, sl]).astype(bf),
                "bq": np.ascontiguousarray(bqkv[0 * C :][sl] * scale).astype(np.float32),
                "bk": np.ascontiguousarray(bqkv[1 * C :][sl]).astype(np.float32),
                "wp": np.ascontiguousarray(Wproj[sl, :]).astype(bf),
                "mask": maskv,
            }
        )
    return in_maps


def gather(outs, bqkv, Wproj, bproj):
    """Sum per-core partials per batch; fold V-bias + proj-bias analytically."""
    bv = bqkv[2 * C :].astype(np.float32)
    bp_eff = (bproj.astype(np.float32) + bv @ Wproj.astype(np.float32)).astype(
        np.float32
    )
    y = np.empty((B, T, C), np.float32)
    for b in range(B):
        acc = outs[b * 4 + 0].astype(np.float32).copy()
        for hg in range(1, 4):
            acc += outs[b * 4 + hg]
        y[b] = acc + bp_eff[None, :]
    return y


def kernel(x, Wqkv, bqkv, Wproj, bproj):
    global LAST_RESULTS
    x = np.asarray(x, dtype=np.float32)
    Wqkv = np.asarray(Wqkv, dtype=np.float32)
    bqkv = np.asarray(bqkv, dtype=np.float32)
    Wproj = np.asarray(Wproj, dtype=np.float32)
    bproj = np.asarray(bproj, dtype=np.float32)

    nc = build()
    in_maps = make_in_maps(x, Wqkv, bqkv, Wproj)
    try:
        res = bass_utils.run_bass_kernel_spmd(
            nc,
            in_maps,
            core_ids=list(range(8)),
            trace=TRACE,
            **TRACE_KWARGS,
        )
    except Exception:
        if not TRACE:
            raise
        import traceback

        traceback.print_exc()
        print("traced run failed; retrying without trace", file=sys.stderr)
        res = bass_utils.run_bass_kernel_spmd(nc, in_maps, core_ids=list(range(8)))
    LAST_RESULTS = res
    outs = [res.results[c]["out"] for c in range(8)]
    return gather(outs, bqkv, Wproj, bproj)
